# revision 1
# baseline (speedup 1.0000x reference)
# Trainium2 Bass kernel for CrossSpeakerAttention.
#
# Per-core (data-parallel over batch B=8 across 8 NeuronCores):
#   X = delta_u[b]  (T=1024, PD=512), heads H=8, D=64
#   token i attends to tokens j with  j < i  AND  spk[j] != spk[i]  AND valid[j].
#   out = softmax(QK^T/8 masked) V, concat heads, @ Wo.T + bo; fully-masked
#   rows produce exactly bo.
#
# Layout strategy (all matmuls run as float32r, ~tf32 precision):
#   - scores are computed TRANSPOSED: ST[j, i] = K_h Q_h^T, so that the
#     post-softmax weights are already in [j (partitions), i (free)] layout for
#     the O^T = V'^T E matmul (no transposes anywhere).
#   - speaker/valid masking is folded into the QK matmul as 5 extra
#     contraction rows (rank-5 additive -BIG bias), run concurrently with the
#     K=64 head matmul on a disjoint PE row-strip (tile_position).
#   - causal masking: upper tiles are skipped entirely; diagonal-straddling
#     subtiles are zeroed post-exp with gpsimd affine_select (exact zeros).
#   - softmax denominators ride along as a 65th "ones" column of V'; the
#     normalization is deferred: 1/denom (with +eps flooring so fully-masked
#     rows collapse to ~0) is broadcast across partitions with gpsimd and
#     applied by the PSUM->SBUF copy of O^T.

import os
import sys
import numpy as np

sys.path.insert(0, "/opt/trn_rl_repo")

_F = lambda name: os.environ.get(name, "1") == "1"   # feature bisect knobs

B, T, PD, H, D = 8, 1024, 512, 8, 64
NT = T // 128          # 8 t/j tiles
NC_ = PD // 128        # 4 contraction tiles for projections
NIB = T // 512         # 2 i-blocks
BIG = 480.0            # additive mask before the 1/8 score scale -> -60
THR = 1e-12            # denominator floor
N_CORES = 8

_CACHE = {}


def _build():
    import concourse.bass as bass
    import concourse.mybir as mybir
    import concourse.tile as tile
    from concourse import bacc

    f32 = mybir.dt.float32
    f32r = mybir.dt.float32r
    AF = mybir.ActivationFunctionType
    OP = mybir.AluOpType

    nc = bacc.Bacc("TRN2", target_bir_lowering=False, debug=False)

    XT_d = nc.dram_tensor("XT", [PD, T], f32, kind="ExternalInput")
    WqT_d = nc.dram_tensor("WqT", [PD, PD], f32, kind="ExternalInput")
    WkT_d = nc.dram_tensor("WkT", [PD, PD], f32, kind="ExternalInput")
    WvT_d = nc.dram_tensor("WvT", [PD, PD], f32, kind="ExternalInput")
    WoT_d = nc.dram_tensor("WoT", [PD, PD], f32, kind="ExternalInput")
    OHQ_d = nc.dram_tensor("OHQ", [128, T], f32, kind="ExternalInput")
    OHK_d = nc.dram_tensor("OHK", [128, T], f32, kind="ExternalInput")
    BO_d = nc.dram_tensor("BO", [1, PD], f32, kind="ExternalInput")
    Y_d = nc.dram_tensor("Y", [T, PD], f32, kind="ExternalOutput")
    DBG = _F("DBG0") and False or os.environ.get("DBG") == "1"
    if DBG:
        dbg = {
            "D_QT0": nc.dram_tensor("D_QT0", [128, T], f32, kind="ExternalOutput"),
            "D_KT0": nc.dram_tensor("D_KT0", [128, T], f32, kind="ExternalOutput"),
            "D_VS0": nc.dram_tensor("D_VS0", [128, 528], f32, kind="ExternalOutput"),
            "D_S2J0": nc.dram_tensor("D_S2J0", [128, 1024], f32, kind="ExternalOutput"),
            "D_E2J0": nc.dram_tensor("D_E2J0", [128, 1024], f32, kind="ExternalOutput"),
            "D_E2J1": nc.dram_tensor("D_E2J1", [128, 1024], f32, kind="ExternalOutput"),
            "D_OPS0": nc.dram_tensor("D_OPS0", [65, 512], f32, kind="ExternalOutput"),
            "D_RCP": nc.dram_tensor("D_RCP", [64, 512], f32, kind="ExternalOutput"),
            "D_RBC0": nc.dram_tensor("D_RBC0", [64, 512], f32, kind="ExternalOutput"),
            "D_RBC1": nc.dram_tensor("D_RBC1", [64, 512], f32, kind="ExternalOutput"),
        }

    with tile.TileContext(nc) as tc:
        import contextlib

        with contextlib.ExitStack() as ctx:
            const = ctx.enter_context(tc.tile_pool(name="const", bufs=1))

            # ---- persistent SBUF tensors (f32r so they can feed matmuls) ----
            xt = const.tile([128, NC_, T], f32r, tag="xt")
            wq = const.tile([128, NC_, PD], f32r, tag="wq")
            wk = const.tile([128, NC_, PD], f32r, tag="wk")
            wv = const.tile([128, NC_, PD], f32r, tag="wv")
            wo = const.tile([128, NC_, PD], f32r, tag="wo")
            ohq = const.tile([128, T], f32r, tag="ohq")
            ohk = const.tile([128, T], f32r, tag="ohk")
            bo = const.tile([1, PD], f32r, tag="bo")
            qt = [const.tile([128, T], f32r, tag=f"qt{p}", name=f"qt{p}") for p in range(4)]
            kt = [const.tile([128, T], f32r, tag=f"kt{p}", name=f"kt{p}") for p in range(4)]
            ot = [const.tile([128, T], f32r, tag=f"ot{p}", name=f"ot{p}") for p in range(4)]
            # V' per j-tile: 8 heads x (64 V cols + ones col + pad) = 528
            vs = [const.tile([128, H, 66], f32r, tag=f"vs{t}", name=f"vs{t}") for t in range(NT)]
            ones_f = const.tile([128, 8], f32, tag="ones_f")

            nc.sync.dma_start(
                xt[:], XT_d.ap().bitcast(f32r).rearrange("(o p) t -> p o t", p=128)
            )
            for w_sb, w_d in ((wq, WqT_d), (wk, WkT_d), (wv, WvT_d), (wo, WoT_d)):
                nc.sync.dma_start(
                    w_sb[:], w_d.ap().bitcast(f32r).rearrange("(o p) n -> p o n", p=128)
                )
            nc.sync.dma_start(ohq[:], OHQ_d.ap().bitcast(f32r))
            nc.sync.dma_start(ohk[:], OHK_d.ap().bitcast(f32r))
            nc.sync.dma_start(bo[:], BO_d.ap().bitcast(f32r))
            nc.vector.memset(ones_f[:], 1.0)

            # ---------------- projections ----------------
            with tc.tile_pool(name="pj", bufs=2, space="PSUM") as pj:
                # Q^T and K^T: out [o-tile(128), t-block(512)]
                for w_sb, dst in ((wq, qt), (wk, kt)):
                    for m in range(4):          # o-tile -> head pair m
                        for tb in range(NIB):
                            ps = pj.tile([128, 512], f32, tag="pj")
                            for k in range(NC_):
                                nc.tensor.matmul(
                                    ps[:],
                                    w_sb[:, k, 128 * m : 128 * (m + 1)],
                                    xt[:, k, 512 * tb : 512 * (tb + 1)],
                                    start=(k == 0),
                                    stop=(k == NC_ - 1),
                                )
                            nc.vector.tensor_copy(
                                dst[m][:, 512 * tb : 512 * (tb + 1)], ps[:]
                            )
                # V: out [t-tile(128), o(512)] -> strided into vs + ones col
                for t in range(NT):
                    ps = pj.tile([128, 512], f32, tag="pj")
                    for k in range(NC_):
                        nc.tensor.matmul(
                            ps[:],
                            xt[:, k, 128 * t : 128 * (t + 1)],
                            wv[:, k, :],
                            start=(k == 0),
                            stop=(k == NC_ - 1),
                        )
                    nc.vector.tensor_copy(
                        vs[t][:, :, 0:64], ps[:].rearrange("p (h d) -> p h d", d=64)
                    )
                    nc.vector.tensor_copy(vs[t][:, :, 64:65], ones_f[:, :, None])

            if DBG:
                dtmp = const.tile([128, 1024], f32, tag="dtmp")
                nc.vector.tensor_copy(dtmp[:], qt[0][:].bitcast(f32))
                nc.sync.dma_start(dbg["D_QT0"].ap(), dtmp[:])
                dtmp2 = const.tile([128, 1024], f32, tag="dtmp2")
                nc.vector.tensor_copy(dtmp2[:], kt[0][:].bitcast(f32))
                nc.sync.dma_start(dbg["D_KT0"].ap(), dtmp2[:])
                dtmp3 = const.tile([128, 528], f32, tag="dtmp3")
                nc.vector.tensor_copy(dtmp3[:], vs[0][:].rearrange("p h d -> p (h d)").bitcast(f32))
                nc.sync.dma_start(dbg["D_VS0"].ap(), dtmp3[:])
            # ---------------- attention ----------------
            with tc.tile_pool(name="s2", bufs=2, space="PSUM") as s2p, \
                 tc.tile_pool(name="ov", bufs=4, space="PSUM") as ovp, \
                 tc.tile_pool(name="esb", bufs=4) as esb, \
                 tc.tile_pool(name="nrm", bufs=4) as nrm:
                for ib in range(NIB):
                    i0 = 512 * ib
                    kept = [jt for jt in range(NT) if 128 * jt - i0 < 512]
                    for pr in range(4):             # head pair -> qt/kt tile pr
                        o_ps = [ovp.tile([65, 512], f32, tag="ov", name=f"ov{ib}_{pr}_{_}") for _ in range(2)]
                        for jt in kept:
                            c0 = min(max(128 * jt - i0, 0), 512)
                            ce = min(c0, 256)   # fp32r wants moving dim >= 256
                            first = jt == kept[0]
                            s2 = s2p.tile([128, 1024], f32, tag="s2")
                            e2 = esb.tile([128, 1024], f32r, tag="e2")
                            for hl in range(2):
                                lo, hi = 64 * hl, 64 * hl + 64
                                ob, oe = 512 * hl + ce, 512 * hl + 512
                                nc.tensor.matmul(
                                    s2[:, ob:oe],
                                    kt[pr][lo:hi, 128 * jt : 128 * (jt + 1)],
                                    qt[pr][lo:hi, i0 + ce : i0 + 512],
                                    start=True,
                                    stop=not _F("K5"),
                                    tile_position=(64 * hl, 0),
                                )
                                # rank-5 speaker/valid bias on a disjoint row strip
                                oh_lo = 64 - 64 * hl
                                if _F("K5"):
                                    nc.tensor.matmul(
                                        s2[:, ob:oe],
                                        ohk[oh_lo : oh_lo + 5, 128 * jt : 128 * (jt + 1)],
                                        ohq[oh_lo : oh_lo + 5, i0 + ce : i0 + 512],
                                        start=False,
                                        stop=True,
                                        tile_position=(oh_lo, 0),
                                    )
                            # exp over both heads (strided): [128, 2, 512-ce]
                            nc.scalar.activation(
                                e2[:].rearrange("p (h n) -> p h n", n=512)[
                                    :, :, ce:512
                                ],
                                s2[:].rearrange("p (h n) -> p h n", n=512)[
                                    :, :, ce:512
                                ],
                                AF.Exp if _F("EXP") else AF.Copy,
                                scale=0.125,
                            )
                            if DBG and ib == 0 and pr == 0 and jt == 0:
                                dS = nrm.tile([128, 1024], f32, tag="dS", name="dS")
                                nc.vector.tensor_copy(dS[:], s2[:])
                                nc.sync.dma_start(dbg["D_S2J0"].ap(), dS[:])
                            # causal zeroing on [ce, c0+128) columns (diag region)
                            if 128 * jt >= i0 and _F("ASEL"):
                                width = (c0 - ce) + 128
                                for hl in range(2):
                                    sl = e2[:, 512 * hl + ce : 512 * hl + ce + width]
                                    nc.gpsimd.affine_select(
                                        sl,
                                        sl,
                                        pattern=[[1, width]],
                                        base=ce - c0,
                                        channel_multiplier=-1,
                                        compare_op=OP.is_gt,
                                        fill=0.0,
                                    )
                            if DBG and ib == 0 and pr == 0 and jt in (0, 1):
                                dE = nrm.tile([128, 1024], f32, tag="dE", name=f"dE{jt}")
                                nc.vector.tensor_copy(dE[:], e2[:].bitcast(f32))
                                nc.sync.dma_start(dbg[f"D_E2J{jt}"].ap(), dE[:])
                            for hl in range(2):
                                h = 2 * pr + hl
                                nc.tensor.matmul(
                                    o_ps[hl][:, ce:512],
                                    vs[jt][:, h, 0:65],
                                    e2[:, 512 * hl + ce : 512 * hl + 512],
                                    start=first,
                                    stop=(jt == kept[-1]),
                                )
                        # ---- normalization ----
                        if DBG and ib == 0 and pr == 0:
                            dO = nrm.tile([65, 512], f32, tag="dO", name="dO")
                            nc.vector.tensor_copy(dO[:], o_ps[0][:])
                            nc.sync.dma_start(dbg["D_OPS0"].ap(), dO[:])
                        coll = nrm.tile([64, 512], f32, tag="coll")
                        rcp = nrm.tile([64, 512], f32, tag="rcp")
                        nc.vector.memset(coll[:], 1.0)
                        for hl in range(2):
                            nc.scalar.copy(
                                coll[32 * hl : 32 * hl + 1, :], o_ps[hl][64:65, :]
                            )
                        nc.vector.tensor_scalar(
                            coll[:], coll[:], THR, None, OP.max
                        )
                        nc.vector.reciprocal(rcp[:], coll[:])
                        for hl in range(2):
                            rbc = nrm.tile([64, 512], f32, tag="rbc")
                            if hl == 0:
                                rsrc = rcp[0:1, :]
                            else:
                                # partition_broadcast only reads partition 0:
                                # shift head B's row down first
                                rtmp = nrm.tile([1, 512], f32, tag="rtmp")
                                nc.vector.tensor_copy(rtmp[0:1, :], rcp[32:33, :])
                                rsrc = rtmp[0:1, :]
                            nc.gpsimd.partition_broadcast(rbc[:], rsrc)
                            if DBG and ib == 0 and pr == 0:
                                if hl == 0:
                                    nc.sync.dma_start(dbg["D_RCP"].ap(), rcp[:])
                                nc.sync.dma_start(dbg[f"D_RBC{hl}"].ap(), rbc[:])
                            if _F("STT"):
                                nc.vector.scalar_tensor_tensor(
                                    ot[pr][64 * hl : 64 * hl + 64, i0 : i0 + 512],
                                    o_ps[hl][0:64, :],
                                    1.0,
                                    rbc[:],
                                    OP.mult,
                                    OP.mult,
                                )
                            else:
                                nc.vector.tensor_copy(
                                    ot[pr][64 * hl : 64 * hl + 64, i0 : i0 + 512],
                                    o_ps[hl][0:64, :],
                                )

            # ---------------- output projection ----------------
            with tc.tile_pool(name="fp", bufs=2, space="PSUM") as fp, \
                 tc.tile_pool(name="ysb", bufs=2) as ysb:
                for t in range(NT):
                    ps = fp.tile([128, 512], f32, tag="fp")
                    for k in range(NC_):
                        nc.tensor.matmul(
                            ps[:],
                            ot[k][:, 128 * t : 128 * (t + 1)],
                            wo[:, k, :],
                            start=(k == 0),
                            stop=False,
                        )
                    # + bo via ones-row (OHQ row 4 is all ones) x bo
                    nc.tensor.matmul(
                        ps[:],
                        ohq[0:1, 128 * t : 128 * (t + 1)],
                        bo[:],
                        start=False,
                        stop=True,
                    )
                    y = ysb.tile([128, 512], f32, tag="y")
                    nc.vector.tensor_copy(y[:], ps[:])
                    nc.sync.dma_start(Y_d.ap()[128 * t : 128 * (t + 1), :], y[:])

    nc.compile()
    return nc


def _prep_core(b, delta_u, speaker_ids, valid_mask, WqT, WkT, WvT, WoT, bo):
    XT = np.ascontiguousarray(delta_u[b].T.astype(np.float32))
    spk = np.asarray(speaker_ids[b]).astype(np.int64)
    valid = np.asarray(valid_mask[b]).astype(np.float32)
    oh = np.zeros((4, T), dtype=np.float32)
    for s in range(4):
        oh[s] = (spk == s).astype(np.float32)
    OHQ = np.zeros((128, T), dtype=np.float32)
    OHK = np.zeros((128, T), dtype=np.float32)
    for base in (0, 64):
        OHQ[base] = 1.0
        OHQ[base + 1 : base + 5] = oh
        OHK[base] = -BIG * (1.0 - valid)
        OHK[base + 1 : base + 5] = -BIG * oh
    return {
        "XT": XT,
        "WqT": WqT,
        "WkT": WkT,
        "WvT": WvT,
        "WoT": WoT,
        "OHQ": OHQ,
        "OHK": OHK,
        "BO": bo.reshape(1, PD).astype(np.float32),
    }


def kernel(**inputs) -> np.ndarray:
    from concourse.bass_utils import run_bass_kernel_spmd

    if "nc" not in _CACHE:
        _CACHE["nc"] = _build()
    nc = _CACHE["nc"]

    delta_u = np.asarray(inputs["delta_u"], dtype=np.float32)
    speaker_ids = np.asarray(inputs["speaker_ids"])
    valid_mask = np.asarray(inputs["valid_mask"])
    Wq = np.asarray(inputs["Wq"], dtype=np.float32)
    Wk = np.asarray(inputs["Wk"], dtype=np.float32)
    Wv = np.asarray(inputs["Wv"], dtype=np.float32)
    Wo = np.asarray(inputs["Wo"], dtype=np.float32)
    bo = np.asarray(inputs["bo"], dtype=np.float32)

    WqT = np.ascontiguousarray(Wq.T)
    WkT = np.ascontiguousarray(Wk.T)
    WvT = np.ascontiguousarray(Wv.T)
    WoT = np.ascontiguousarray(Wo.T)

    in_maps = [
        _prep_core(b, delta_u, speaker_ids, valid_mask, WqT, WkT, WvT, WoT, bo)
        for b in range(N_CORES)
    ]
    _CACHE["last_in_maps"] = in_maps
    res = run_bass_kernel_spmd(nc, in_maps, list(range(N_CORES)))
    out = np.stack([res.results[b]["Y"] for b in range(N_CORES)], axis=0)
    return out.astype(np.float32)


if __name__ == "__main__":
    rng = np.random.default_rng(0)
    ins = {
        "delta_u": rng.standard_normal((B, T, PD), dtype=np.float32),
        "speaker_ids": rng.integers(0, 4, size=(B, T)),
        "valid_mask": np.ones((B, T), dtype=bool),
        "Wq": rng.standard_normal((PD, PD), dtype=np.float32) * PD**-0.5,
        "Wk": rng.standard_normal((PD, PD), dtype=np.float32) * PD**-0.5,
        "Wv": rng.standard_normal((PD, PD), dtype=np.float32) * PD**-0.5,
        "Wo": rng.standard_normal((PD, PD), dtype=np.float32) * PD**-0.5,
        "bo": np.zeros((PD,), dtype=np.float32),
    }
    y = kernel(**ins)
    print("kernel ran, out shape", y.shape)



# revision 26
# speedup vs baseline: 1.2443x; 1.2443x over previous
# Trainium2 Bass kernel for CrossSpeakerAttention.
#
# Per-core (data-parallel over batch B=8 across 8 NeuronCores):
#   X = delta_u[b]  (T=1024, PD=512), heads H=8, D=64
#   token i attends to tokens j with  j < i  AND  spk[j] != spk[i]  AND valid[j].
#   out = softmax(QK^T/8 masked) V, concat heads, @ Wo.T + bo; fully-masked
#   rows produce exactly bo.
#
# v2 layout strategy (all matmuls float32r):
#   - scores TRANSPOSED: ST[j, i] = K_h Q_h^T so post-softmax weights are in
#     [j (partitions), i (free)] layout for the O^T = V'^T E matmul.
#   - speaker/valid masking rides INSIDE the score matmul: per-head augmented
#     Q^T/K^T tiles of 69 partitions (64 head dims + 5 one-hot bias rows), so
#     the rank-5 additive -BIG bias costs zero extra PE time (matmul cost
#     depends only on the moving dim, not the contraction rows).
#   - causal: upper tiles skipped; diagonal subtiles zeroed post-exp with
#     gpsimd affine_select (exact zeros).
#   - softmax denominators ride as a 65th "ones" column of V'; normalization
#     is a deferred divide: floor(denom, THR) -> partition_broadcast ->
#     (O / denom) on the PSUM->SBUF copy. Fully-masked rows collapse to ~0,
#     then + bo from the output projection gives exactly bo.
#   - output bias via a host-broadcast BO tile folded into the y copy (no
#     bias matmul).
#   - input DMAs are chunked per 128-row group so the first projection matmul
#     starts ~2us in; Q/K/V projections for heads 2..7 are software-pipelined
#     into the attention phase as PE filler between score/AV matmuls.

import os
import sys
import numpy as np

sys.path.insert(0, "/opt/trn_rl_repo")

B, T, PD, H, D = 8, 1024, 512, 8, 64
NT = T // 128          # 8 j tiles
NC_ = PD // 128        # 4 contraction tiles for projections
NIB = T // 512         # 2 i-blocks
BIG = 480.0            # additive mask before the 1/8 score scale -> -60
THR = 1e-12            # denominator floor
N_CORES = 8

_CACHE = {}


def _build():
    import concourse.bass as bass
    import concourse.mybir as mybir
    import concourse.tile as tile
    from concourse import bacc

    f32 = mybir.dt.float32
    f32r = mybir.dt.float32r
    bf16 = mybir.dt.bfloat16
    AF = mybir.ActivationFunctionType
    OP = mybir.AluOpType

    nc = bacc.Bacc("TRN2", target_bir_lowering=False, debug=False)

    XT_d = nc.dram_tensor("XT", [PD, T], bf16, kind="ExternalInput")
    WQK_d = nc.dram_tensor("WQK", [2 * PD, PD], bf16, kind="ExternalInput")
    WvT_d = nc.dram_tensor("WvT", [PD, PD], bf16, kind="ExternalInput")
    WoT_d = nc.dram_tensor("WoT", [PD, PD], bf16, kind="ExternalInput")
    OHQK_d = nc.dram_tensor("OHQK", [5, 2 * H * T], bf16, kind="ExternalInput")
    BOB_d = nc.dram_tensor("BOB", [128, PD], f32, kind="ExternalInput")
    Y_d = nc.dram_tensor("Y", [T, PD], f32, kind="ExternalOutput")

    with tile.TileContext(nc) as tc:
        import contextlib

        with contextlib.ExitStack() as ctx:
            const = ctx.enter_context(tc.tile_pool(name="const", bufs=1))

            # ---- persistent SBUF tensors ----
            xt = const.tile([128, NC_, T], bf16, tag="xt")
            wqk = const.tile([128, 2, NC_, PD], bf16, tag="wqk")
            wv = const.tile([128, NC_, PD], bf16, tag="wv")
            wo = const.tile([128, NC_, PD], bf16, tag="wo")
            bob = const.tile([128, PD], f32, tag="bob")
            # per-head augmented Q^T/K^T: rows 0:64 head dims, 64:69 bias
            # rows; [w, h] indexes Q (w=0) / K (w=1) per head.
            qkth = const.tile([69, 2, H, T], bf16, tag="qkth")
            ot = [const.tile([128, T], bf16, tag=f"ot{p}", name=f"ot{p}") for p in range(4)]
            # V' per j-tile: 8 heads x (64 V cols + ones col + pad) = 528
            vs = [const.tile([128, H, 66], bf16, tag=f"vs{t}", name=f"vs{t}") for t in range(NT)]
            ones_f = const.tile([128, 8], f32, tag="ones_f")

            # ---- input DMAs, few and ordered so compute starts early ----
            nc.sync.dma_start(
                xt[:, 0:2, :],
                XT_d.ap()[0:256, :].rearrange("(o p) t -> p o t", p=128),
            )
            nc.sync.dma_start(
                wqk[:, 0, :, :],
                WQK_d.ap()[0:512, :].rearrange("(o p) n -> p o n", p=128),
            )
            nc.sync.dma_start(
                xt[:, 2:4, :],
                XT_d.ap()[256:512, :].rearrange("(o p) t -> p o t", p=128),
            )
            nc.sync.dma_start(
                wqk[:, 1, :, :],
                WQK_d.ap()[512:1024, :].rearrange("(o p) n -> p o n", p=128),
            )
            nc.sync.dma_start(
                qkth[64:69, :, :, :],
                OHQK_d.ap().rearrange("p (w h t) -> p w h t", w=2, h=H),
            )
            nc.sync.dma_start(
                wv[:], WvT_d.ap().rearrange("(o p) n -> p o n", p=128)
            )
            nc.sync.dma_start(
                wo[:], WoT_d.ap().rearrange("(o p) n -> p o n", p=128)
            )
            nc.sync.dma_start(bob[:], BOB_d.ap())
            nc.vector.memset(ones_f[:], 1.0)

            def proj_qk(pool, tag, m, engines):
                # one head-pair o-tile of Q^T and K^T -> per-head aug tiles
                for w in range(2):
                    for tb in range(NIB):
                        ps = pool.tile([128, 512], f32, tag=tag)
                        for k in range(NC_):
                            nc.tensor.matmul(
                                ps[:],
                                wqk[:, w, k, 128 * m : 128 * (m + 1)],
                                xt[:, k, 512 * tb : 512 * (tb + 1)],
                                start=(k == 0),
                                stop=(k == NC_ - 1),
                            )
                        for hl in range(2):
                            eng = engines[(w * 2 + tb + hl) % len(engines)]
                            dst = qkth[0:64, w, 2 * m + hl, 512 * tb : 512 * (tb + 1)]
                            if eng is nc.scalar:
                                eng.copy(dst, ps[64 * hl : 64 * hl + 64, :])
                            else:
                                eng.tensor_copy(dst, ps[64 * hl : 64 * hl + 64, :])

            def proj_v(pool, tag, t):
                ps = pool.tile([128, 512], f32, tag=tag)
                for k in range(NC_):
                    nc.tensor.matmul(
                        ps[:],
                        xt[:, k, 128 * t : 128 * (t + 1)],
                        wv[:, k, :],
                        start=(k == 0),
                        stop=(k == NC_ - 1),
                    )
                nc.vector.tensor_copy(
                    vs[t][:, :, 0:64], ps[:].rearrange("p (h d) -> p h d", d=64)
                )
                nc.vector.tensor_copy(vs[t][:, :, 64:65], ones_f[:, :, None])

            # ---- pre-attention: Q0/K0 head pair + V tiles for ib0 ----
            with tc.tile_pool(name="pjpre", bufs=2, space="PSUM") as pjpre:
                proj_qk(pjpre, "pj", 0, [nc.vector, nc.scalar])
                for t in range(4):
                    proj_v(pjpre, "pj", t)

            # ---------------- attention ----------------
            # per-head score tiles (1 PSUM bank each) allow a 4-deep ring and
            # a 3-step software pipeline: AV(step) issues 3 scores later, so
            # the ~2us scores->exp->affine->AV chain latency stays hidden.
            # Remaining projections run as PE filler drawing from the same
            # score-tile ring (pre-attention pool is closed by now).
            DEPTH = 3
            with tc.tile_pool(name="s2", bufs=4, space="PSUM") as s2p, \
                 tc.tile_pool(name="ov", bufs=4, space="PSUM") as ovp, \
                 tc.tile_pool(name="esb", bufs=6) as esb, \
                 tc.tile_pool(name="nrm", bufs=4) as nrm:
                fillers = []
                for t in range(4, NT):
                    fillers.append(lambda t=t: proj_v(s2p, "s2", t))
                for m in range(1, 4):
                    fillers.append(
                        lambda m=m: proj_qk(s2p, "s2", m, [nc.vector])
                    )
                fill_i = 0

                def pop_filler():
                    nonlocal fill_i
                    if fill_i < len(fillers):
                        fillers[fill_i]()
                        fill_i += 1

                for pr in range(4):             # head pair
                    for ib in range(NIB):
                        i0 = 512 * ib
                        kept = [jt for jt in range(NT) if 128 * jt - i0 < 512]
                        o_ps = [
                            ovp.tile([65, 512], f32, tag="ov", name=f"ov{ib}_{pr}_{_}")
                            for _ in range(2)
                        ]

                        def av(jt, hl, e2):
                            c0 = min(max(128 * jt - i0, 0), 512)
                            nc.tensor.matmul(
                                o_ps[hl][:, c0:512],
                                vs[jt][:, 2 * pr + hl, 0:65],
                                e2[:, c0:512],
                                start=(jt == kept[0]),
                                stop=(jt == kept[-1]),
                            )

                        steps = [(jt, hl) for jt in kept for hl in range(2)]
                        pend = []
                        for idx, (jt, hl) in enumerate(steps):
                            c0 = min(max(128 * jt - i0, 0), 512)
                            h = 2 * pr + hl
                            s2 = s2p.tile([128, 512], f32, tag="s2")
                            e2 = esb.tile([128, 512], bf16, tag="e2")
                            nc.tensor.matmul(
                                s2[:, c0:512],
                                qkth[0:69, 1, h, 128 * jt : 128 * (jt + 1)],
                                qkth[0:69, 0, h, i0 + c0 : i0 + 512],
                                start=True,
                                stop=True,
                            )
                            if len(pend) >= DEPTH:
                                av(*pend.pop(0))
                            if idx % 5 == 2:
                                pop_filler()
                            nc.scalar.activation(
                                e2[:, c0:512], s2[:, c0:512], AF.Exp, scale=0.125
                            )
                            # causal zeroing on [c0, c0+128) (diag region)
                            if 128 * jt >= i0:
                                sl = e2[:, c0 : c0 + 128]
                                nc.gpsimd.affine_select(
                                    sl,
                                    sl,
                                    pattern=[[1, 128]],
                                    base=0,
                                    channel_multiplier=-1,
                                    compare_op=OP.is_gt,
                                    fill=0.0,
                                )
                            pend.append((jt, hl, e2))
                        for p in pend:
                            av(*p)
                        # ---- normalization: ot = O * (1/max(denom, THR)) ----
                        for hl in range(2):
                            rr = nrm.tile([1, 512], f32, tag=f"rr{hl}", name=f"rr{hl}")
                            nc.vector.tensor_scalar(
                                rr[:], o_ps[hl][64:65, :], THR, None, OP.max
                            )
                            nc.vector.reciprocal(rr[:], rr[:])
                            rbc = nrm.tile([64, 512], f32, tag=f"rbc{hl}", name=f"rbc{hl}")
                            nc.gpsimd.partition_broadcast(rbc[:], rr[0:1, :])
                            nc.vector.scalar_tensor_tensor(
                                ot[pr][64 * hl : 64 * hl + 64, i0 : i0 + 512],
                                o_ps[hl][0:64, :],
                                1.0,
                                rbc[:],
                                OP.mult,
                                OP.mult,
                            )

            # ---------------- output projection ----------------
            with tc.tile_pool(name="pjo", bufs=2, space="PSUM") as pj, \
                 tc.tile_pool(name="ysb", bufs=2) as ysb:
                for tp in range(NT // 2):
                    y = ysb.tile([128, 2, 512], f32, tag="y")
                    for ti in range(2):
                        t = 2 * tp + ti
                        ps = pj.tile([128, 512], f32, tag="pj")
                        for k in range(NC_):
                            nc.tensor.matmul(
                                ps[:],
                                ot[k][:, 128 * t : 128 * (t + 1)],
                                wo[:, k, :],
                                start=(k == 0),
                                stop=(k == NC_ - 1),
                            )
                        nc.vector.tensor_tensor(y[:, ti, :], ps[:], bob[:], OP.add)
                    nc.sync.dma_start(
                        Y_d.ap()[256 * tp : 256 * (tp + 1), :].rearrange(
                            "(t p) n -> p t n", p=128
                        ),
                        y[:],
                    )

    nc.compile()
    return nc


def _prep_core(b, delta_u, speaker_ids, valid_mask, WqT, WkT, WvT, WoT, bo):
    import ml_dtypes

    bf = ml_dtypes.bfloat16
    XT = np.ascontiguousarray(delta_u[b].T.astype(bf))
    spk = np.asarray(speaker_ids[b]).astype(np.int64)
    valid = np.asarray(valid_mask[b]).astype(np.float32)
    oh = np.zeros((4, T), dtype=np.float32)
    for s in range(4):
        oh[s] = (spk == s).astype(np.float32)
    OHQ = np.concatenate([np.ones((1, T), np.float32), oh], axis=0)
    OHK = np.concatenate([(-BIG * (1.0 - valid)).reshape(1, T), -BIG * oh], axis=0)
    OHQK = np.concatenate([np.tile(OHQ, (1, H)), np.tile(OHK, (1, H))], axis=1)
    WQK = np.concatenate([WqT, WkT], axis=0)
    BOB = np.tile(bo.reshape(1, PD).astype(np.float32), (128, 1))
    return {
        "XT": XT,
        "WQK": np.ascontiguousarray(WQK).astype(bf),
        "WvT": WvT.astype(bf),
        "WoT": WoT.astype(bf),
        "OHQK": np.ascontiguousarray(OHQK).astype(bf),
        "BOB": BOB,
    }


def kernel(**inputs) -> np.ndarray:
    from concourse.bass_utils import run_bass_kernel_spmd

    if "nc" not in _CACHE:
        _CACHE["nc"] = _build()
    nc = _CACHE["nc"]

    delta_u = np.asarray(inputs["delta_u"], dtype=np.float32)
    speaker_ids = np.asarray(inputs["speaker_ids"])
    valid_mask = np.asarray(inputs["valid_mask"])
    Wq = np.asarray(inputs["Wq"], dtype=np.float32)
    Wk = np.asarray(inputs["Wk"], dtype=np.float32)
    Wv = np.asarray(inputs["Wv"], dtype=np.float32)
    Wo = np.asarray(inputs["Wo"], dtype=np.float32)
    bo = np.asarray(inputs["bo"], dtype=np.float32)

    WqT = np.ascontiguousarray(Wq.T)
    WkT = np.ascontiguousarray(Wk.T)
    WvT = np.ascontiguousarray(Wv.T)
    WoT = np.ascontiguousarray(Wo.T)

    in_maps = [
        _prep_core(b, delta_u, speaker_ids, valid_mask, WqT, WkT, WvT, WoT, bo)
        for b in range(N_CORES)
    ]
    _CACHE["last_in_maps"] = in_maps
    res = run_bass_kernel_spmd(nc, in_maps, list(range(N_CORES)))
    out = np.stack([res.results[b]["Y"] for b in range(N_CORES)], axis=0)
    return out.astype(np.float32)


if __name__ == "__main__":
    rng = np.random.default_rng(0)
    ins = {
        "delta_u": rng.standard_normal((B, T, PD), dtype=np.float32),
        "speaker_ids": rng.integers(0, 4, size=(B, T)),
        "valid_mask": np.ones((B, T), dtype=bool),
        "Wq": rng.standard_normal((PD, PD), dtype=np.float32) * PD**-0.5,
        "Wk": rng.standard_normal((PD, PD), dtype=np.float32) * PD**-0.5,
        "Wv": rng.standard_normal((PD, PD), dtype=np.float32) * PD**-0.5,
        "Wo": rng.standard_normal((PD, PD), dtype=np.float32) * PD**-0.5,
        "bo": np.zeros((PD,), dtype=np.float32),
    }
    y = kernel(**ins)
    print("kernel ran, out shape", y.shape)
